# revision 2
# baseline (speedup 1.0000x reference)
"""Trainium2 Bass kernel for nn_Attention_85796266705382.

Reference computation (per batch element, b=8, HEAD=8, n=32*32=1024, c=dim=512):
    qkv = x @ w_qkv                      # (n, 1536), per-head interleaved [q|k|v] x 64
    q,k,v per head (n, 64)
    attn = softmax(q @ k.T * 8**-0.5)    # scale uses FULL batch size (reference quirk)
    out  = attn @ v                      # (n, 64) per head -> (n, 512)
    y    = out @ w_out + b_out           # (n, 512)

Sharding: pure data-parallel over batch - one batch element per NeuronCore.

Design (~126us HW; prior session baseline was ~145us):
  * steady state is exp-bound: ScalarE runs 64 x [128,1024] Exp back-to-back
    (~66us floor); rounds process scores for pair p while pair p-1's AV
    matmuls fill the PE idle slack between exp-gated score groups.
  * scores: zero-padded 128-contraction (uniform PE tile mode, 216ns
    steady-state, no LDW-transition exposure). 64-row tiled scores measured
    WORSE (no concurrency: PSUM-slot semaphores stagger the pair).
  * AV results are EVICTED from PSUM to SBUF (f32r [65, 2N] per pair) right
    after accumulation: PSUM banks free deterministically (no DMA in any
    PE-gating path), so the next pair's AV interleaves instead of bursting.
    AV matmuls are front-loaded over scores-jt 0..5 (AV_SPLITS) so the den
    chain completes mid-round.
  * den path: the evicted ones-column row (row 64) is DMA-gathered SBUF->SBUF
    to [128,16], reciprocal'd in ~260ns, bounced via DRAM (bf16) and
    broadcast with a stride-0 partition AP to [64,2,N]. Normalizing mults
    are pure-SBUF DVE ops that gate nothing on the PE side.
  * tail: fused final projection - 4-matmul PSUM accumulation per it-tile
    (waves over psB+psS banks; partials p4=0..2 run during the pair-3 den
    chain), one bias-add per tile evicts straight to bf16, stores alternate
    the sync/scalar HWDGE rings. Output is stored bf16 (halves the 2MB
    drain; ~0.2% extra rounding vs the 2e-2 budget).
  * input loads alternate the two HWDGE rings (sync/scalar) to parallelize
    the ~600ns-per-dma descriptor issue in the preamble.
  * NOTE: do NOT co-schedule the qk-prefetch matmul chunks densely into the
    exp-gated slots (v9 experiment): overlapping all engines saturates an
    SBUF/PSUM-port (or power) limit and EVERY engine slows ~15% uniformly
    (126us -> 146us, reproducible). The two ~1.7us exp gaps per round where
    the 8-MM prefetch blocks run are load-bearing slack.
"""

import numpy as np


def _ensure_paths():
    import sys

    try:
        import concourse.bass  # noqa: F401

        return
    except ImportError:
        pass
    for p in ("/opt/trn_rl_repo", "/root/.axon_site/_ro/trn_rl_repo"):
        if p not in sys.path:
            sys.path.append(p)
    import concourse.bass  # noqa: F401


HEAD = 8
B = 8
N = 1024  # tokens per batch element (32*32)
C = 512  # channels
DIM = 512
DH = 64
SCALE = float(B) ** -0.5  # reference scales by batch size, reproduced faithfully
N_CORES = 8

_CACHE = {}


def _split_excess_waits(nc, mybir, bass_rust):
    """walrus in this container accepts 1 sync wait per instruction (2 for
    EventSemaphore); Tile sometimes attaches more. Hoist the excess onto fresh
    same-engine NoOps inserted just before the over-capacity instruction."""
    n_split = 0
    for fn in nc.m.functions:
        for bb in fn.blocks:
            insts = bb.instructions
            i = 0
            while i < len(insts):
                inst = insts[i]
                si = inst.sync_info
                cap = 2 if isinstance(inst, mybir.InstEventSemaphore) else 1
                if si is not None and len(si.on_wait) > cap:
                    extra = list(si.on_wait[cap:])
                    del si.on_wait[cap:]
                    new_insts = []
                    for k in range(0, len(extra), 2):
                        pair = extra[k : k + 2]
                        nop = mybir.InstEventSemaphore(
                            name=f"{inst.name}_ws{k}", ins=[], outs=[]
                        )
                        nop.engine = inst.engine
                        nop.sync_info = bass_rust.SyncInfo(on_wait=pair, on_update=[])
                        new_insts.append(nop)
                        n_split += 1
                    insts[i:i] = new_insts
                    i += len(new_insts)
                i += 1
    return n_split


def _build():
    if "nc" in _CACHE:
        return _CACHE["nc"]
    _ensure_paths()
    import bass_rust
    import concourse.bass as bass
    import concourse.mybir as mybir
    import concourse.tile as tile

    f32 = mybir.dt.float32
    f32r = mybir.dt.float32r
    bf16 = mybir.dt.bfloat16
    u16 = mybir.dt.uint16
    u32 = mybir.dt.uint32
    Exp = mybir.ActivationFunctionType.Exp
    Copy = mybir.ActivationFunctionType.Copy

    nc = bass.Bass(trn_type="TRN2", target_bir_lowering=False, debug=False)

    xT_d = nc.dram_tensor("xT", [C, N], bf16, kind="ExternalInput").ap()
    wqk_d = nc.dram_tensor("w_qk", [C, 2 * DIM], bf16, kind="ExternalInput").ap()
    wv_d = nc.dram_tensor("w_v", [C, DIM], bf16, kind="ExternalInput").ap()
    wo_d = nc.dram_tensor("w_out", [DIM, DIM], f32r, kind="ExternalInput").ap()
    b_d = nc.dram_tensor("b_out", [DIM], f32r, kind="ExternalInput").ap()
    out_d = nc.dram_tensor("out", [N, DIM], bf16, kind="ExternalOutput").ap()
    # per-pair reciprocal-den bounce slots (all distinct: no WAR hazards)
    rd2_d = nc.dram_tensor("rd2_scratch", [4, 2 * N], bf16).ap()

    with tile.TileContext(nc) as tc:
        with (
            tc.tile_pool(name="wp", bufs=1) as wp,
            tc.tile_pool(name="xp", bufs=1) as xp,
            tc.tile_pool(name="qkp", bufs=1) as qkp,
            tc.tile_pool(name="vpp", bufs=1) as vpp,
            tc.tile_pool(name="ptp", bufs=28) as ptp,
            tc.tile_pool(name="otp", bufs=1) as otp,
            tc.tile_pool(name="evp", bufs=4) as evp,
            tc.tile_pool(name="gp", bufs=3) as gp,
            tc.tile_pool(name="bcp", bufs=3) as bcp,
            tc.tile_pool(name="ysp", bufs=1) as ysp,
            tc.tile_pool(name="obp", bufs=3) as obp,
            tc.tile_pool(name="psS", bufs=2, space="PSUM") as psS,
            tc.tile_pool(name="psB", bufs=4, space="PSUM") as psB,
        ):
            # ---- input loads: first-needed chunks first, alternating the two
            # HWDGE rings (sync / scalar) to parallelize descriptor issue ----
            def eng(i):
                return nc.sync if i % 2 == 0 else nc.scalar

            xT, wqk = [], []
            for ct in range(4):
                xT.append(xp.tile([128, N], bf16, tag=f"xT{ct}", name=f"xT{ct}"))
                wqk.append(
                    wp.tile([128, 2 * DIM], bf16, tag=f"wqk{ct}", name=f"wqk{ct}")
                )
            for ct in range(4):
                eng(ct).dma_start(
                    out=wqk[ct][:, 0:256], in_=wqk_d[ct * 128 : (ct + 1) * 128, 0:256]
                )
                eng(ct + 1).dma_start(
                    out=xT[ct][:], in_=xT_d[ct * 128 : (ct + 1) * 128, :]
                )
            for ct in range(4):
                eng(ct + 1).dma_start(
                    out=wqk[ct][:, 256:1024],
                    in_=wqk_d[ct * 128 : (ct + 1) * 128, 256:1024],
                )
            wv = []
            for ct in range(4):
                t = wp.tile([128, DIM], bf16, tag=f"wv{ct}", name=f"wv{ct}")
                eng(ct).dma_start(out=t[:], in_=wv_d[ct * 128 : (ct + 1) * 128, :])
                wv.append(t)

            def xTs(ct, a, b):
                return xT[ct][:, a:b]

            def wqks(ct, a, b):
                return wqk[ct][:, a:b]

            # zero-padded K-tiles: kp[par][w] holds head-w's K rows in the
            # same 64-row band as the Q tile layout, other 64 rows ZERO, so
            # score matmuls contract over the full 128 partitions (exact).
            kp = {}
            for par in range(2):
                for w in range(2):
                    t = qkp.tile([128, N], bf16, tag=f"kp{par}{w}", name=f"kp{par}{w}")
                    if w == 0:
                        nc.vector.memset(t[64:128, :].bitcast(u16), 0)
                    else:
                        nc.vector.memset(t[0:64, :].bitcast(u16), 0)
                    kp[(par, w)] = t

            def qk_tile(dt_i):
                """one d-tile of the transposed qk projection -> SBUF bf16.
                Odd d-tiles (K) are written as the zero-padded kp pair."""
                ps = psS.tile([128, N], f32, tag="S", name=f"qkps{dt_i}")
                for ch in range(2):
                    for ct in range(4):
                        nc.tensor.matmul(
                            ps[:, ch * 512 : (ch + 1) * 512],
                            wqks(ct, dt_i * 128, (dt_i + 1) * 128),
                            xTs(ct, ch * 512, (ch + 1) * 512),
                            start=(ct == 0),
                            stop=(ct == 3),
                        )
                if dt_i % 2 == 0:
                    t = qkp.tile([128, N], bf16, tag=f"qk{dt_i}", name=f"qk{dt_i}")
                    nc.vector.tensor_copy(t[:], ps[:])
                    return t
                par = (dt_i // 2) % 2
                nc.vector.tensor_copy(kp[(par, 0)][0:64, :], ps[0:64, :])
                nc.vector.tensor_copy(kp[(par, 1)][64:128, :], ps[64:128, :])
                return par

            def scores_jt(p, jt, QT, par):
                """score matmuls (zero-padded K=128) + exp for one (pair, jt)"""
                sps = [
                    psS.tile([128, N], f32, tag="S", name=f"s_{p}_{jt}_{w}")
                    for w in range(2)
                ]
                for w in range(2):
                    for ch in range(2):
                        nc.tensor.matmul(
                            sps[w][:, ch * 512 : (ch + 1) * 512],
                            kp[(par, w)][:, jt * 128 : (jt + 1) * 128],
                            QT[:, ch * 512 : (ch + 1) * 512],
                            start=True,
                            stop=True,
                        )
                pts = []
                for w in range(2):
                    pt = ptp.tile([128, N], bf16, tag="pt", name=f"pt{p}_{jt}_{w}")
                    nc.scalar.activation(pt[:], sps[w][:], Exp, scale=SCALE)
                    pts.append(pt)
                return pts

            def v_jt(jt):
                """one j-tile of the v projection, 65-pitch + ones column"""
                vt = vpp.tile([128, HEAD, DH + 1], bf16, tag=f"v{jt}", name=f"v{jt}")
                nc.vector.memset(vt[:, :, DH : DH + 1].bitcast(u16), 16256)
                ps = psB.tile([128, 512], f32, tag="B", name=f"vps{jt}")
                for ct in range(4):
                    nc.tensor.matmul(
                        ps[:],
                        xTs(ct, jt * 128, (jt + 1) * 128),
                        wv[ct][:],
                        start=(ct == 0),
                        stop=(ct == 3),
                    )
                nc.vector.tensor_copy(
                    vt[:, :, 0:DH],
                    ps[:].rearrange("p (h e) -> p h e", h=HEAD),
                )
                return vt

            def make_av(p):
                return [
                    [
                        psB.tile([128, 512], f32, tag="B", name=f"av{p}_{w}_{c}")
                        for c in range(2)
                    ]
                    for w in range(2)
                ]

            def av_mm(p, jtAV, w, ch, av, pts):
                nc.tensor.matmul(
                    av[w][ch][0 : DH + 1, :],
                    v_sb[jtAV][:, 2 * p + w, :],
                    pts[jtAV][w][:, ch * 512 : (ch + 1) * 512],
                    start=(jtAV == 0),
                    stop=(jtAV == 7),
                )

            # flattened AV matmul order for one pair (32 mms)
            AV_ORDER = [
                (jtAV, w, ch) for jtAV in range(8) for w in range(2) for ch in range(2)
            ]
            # spread over scores-jt 0..5 so evict/den complete mid-round
            AV_SPLITS = [0, 6, 12, 18, 24, 28, 32]

            def evict(p, av):
                """PSUM -> SBUF eviction of the AV result (+ den row 64).
                Frees the PSUM banks deterministically so the next pair's AV
                can interleave; nothing downstream gates the PE. One [65, 2N]
                tile per pair so the den row is a single contiguous [1, 2N]."""
                ev = evp.tile([DH + 1, 2 * N], f32r, tag="ev", name=f"ev{p}")
                for w in range(2):
                    for ch in range(2):
                        nc.vector.tensor_copy(
                            ev[:, w * N + ch * 512 : w * N + (ch + 1) * 512],
                            av[w][ch][0 : DH + 1, :],
                        )
                return ev

            def den_chain(p, ev):
                """den row -> 1/den broadcast tile: one DMA gather to [128,16],
                tiny reciprocal, one DMA back to DRAM, one stride-0 partition
                broadcast (4D AP deinterleaves the w halves)."""
                gi = gp.tile([128, 16], f32r, tag="gi", name=f"gi{p}")
                nc.sync.dma_start(out=gi[:], in_=ev[64:65, :])
                go = gp.tile([128, 16], bf16, tag="go", name=f"go{p}")
                with nc.allow_low_precision(reason="1/den tolerates bf16"):
                    nc.vector.reciprocal(go[:], gi[:])
                nc.sync.dma_start(out=rd2_d[p : p + 1, :], in_=go[:])
                bc = bcp.tile([64, 2, N], bf16, tag="bc", name=f"bc{p}")
                # flat gather/back round-trip keeps the two w halves contiguous
                bc_src = bass.AP(
                    tensor=rd2_d.tensor, offset=p * 2 * N, ap=[[0, 64], [1, 2 * N]]
                )
                nc.sync.dma_start(out=bc[:], in_=bc_src)
                return bc

            def mults(p, ev, bc, ot):
                """normalizing multiplies, pure SBUF (gates nothing on PE)"""
                for w in range(2):
                    nc.vector.tensor_mul(
                        ot[w * 64 : (w + 1) * 64, :],
                        ev[0:DH, w * N : (w + 1) * N],
                        bc[:, w, :],
                    )

            # ================= round 0: qk pair 0 + scores p0 + v proj =======
            qk = {}
            qk[0] = qk_tile(0)
            par0 = qk_tile(1)

            v_sb = [None] * 8
            pts_prev = [None] * 8
            for jt in range(8):
                pts_prev[jt] = scores_jt(0, jt, qk[0], par0)
                v_sb[jt] = v_jt(jt)
                if jt == 4:
                    qk[2] = qk_tile(2)
                if jt == 6:
                    qk[3] = qk_tile(3)

            ot_tiles = [None] * 4
            av3 = None
            evs3 = None

            # ================= rounds 1-3: scores p | AV p-1 =================
            # AV for pair p-1 is front-loaded over jt 0..5 so its eviction +
            # den DMA chain complete mid-round and the normalizing mults land
            # at round end. Pair 3's own AV additionally starts inside round 3
            # (as its pts stream in) so the tail isn't gated on a full AV pass.
            for p in range(1, 4):
                QT, par = qk[2 * p], qk[2 * p + 1]
                av = make_av(p - 1)
                pts_cur = [None] * 8
                evs = None
                for jt in range(8):
                    pts_cur[jt] = scores_jt(p, jt, QT, par)
                    if jt < 6:
                        for jtAV, w, ch in AV_ORDER[AV_SPLITS[jt] : AV_SPLITS[jt + 1]]:
                            av_mm(p - 1, jtAV, w, ch, av, pts_prev)
                    elif p == 3:
                        # pair-3 AV rides the last exp-gated slots of round 3
                        av3 = av3 if av3 is not None else make_av(3)
                        lo, hi = (0, 2) if jt == 6 else (2, 5)
                        for jtAV in range(lo, hi):
                            for w in range(2):
                                for ch in range(2):
                                    av_mm(3, jtAV, w, ch, av3, pts_cur)
                    if jt == 5:
                        evs = evict(p - 1, av)
                        bc = den_chain(p - 1, evs)
                        if p == 3:
                            # DVE is otherwise idle until evict(3); emitting
                            # the ot2 mults here lets ysub start at round end
                            ot_tiles[2] = otp.tile(
                                [128, N], f32r, tag="ot2", name="ot2"
                            )
                            mults(2, evs, bc, ot_tiles[2])
                    if jt == 4:
                        if p < 3:
                            qk[2 * p + 2] = qk_tile(2 * p + 2)
                        else:
                            # late weight loads (final projection only); kept
                            # off the scalar ring so the exp stream stays fed
                            wo = []
                            for p4 in range(4):
                                t = wp.tile(
                                    [128, DIM], f32r, tag=f"wo{p4}", name=f"wo{p4}"
                                )
                                nc.sync.dma_start(
                                    out=t[:], in_=wo_d[p4 * 128 : (p4 + 1) * 128, :]
                                )
                                wo.append(t)
                            bb_t = wp.tile([128, DIM], f32r, tag="bb", name="bb")
                            b_src = bass.AP(
                                tensor=b_d.tensor,
                                offset=b_d.offset,
                                ap=[[0, 128]] + list(b_d.ap),
                            )
                            nc.sync.dma_start(out=bb_t[:], in_=b_src)
                    if jt == 6 and p < 3:
                        qk[2 * p + 3] = qk_tile(2 * p + 3)
                if p < 3:
                    ot_tiles[p - 1] = otp.tile(
                        [128, N], f32r, tag=f"ot{p - 1}", name=f"ot{p - 1}"
                    )
                    mults(p - 1, evs, bc, ot_tiles[p - 1])
                pts_prev = pts_cur

            # ================= round 4: finish AV p3 | ysub | tail ===========
            for jtAV in range(5, 8):
                for w in range(2):
                    for ch in range(2):
                        av_mm(3, jtAV, w, ch, av3, pts_prev)
            evs3 = evict(3, av3)
            bc3 = den_chain(3, evs3)
            ot3 = otp.tile([128, N], f32r, tag="ot3", name="ot3")
            ot_tiles[3] = ot3
            mults(3, evs3, bc3, ot3)

            # ========= tail: fused final projection (waves over PSUM) ======
            # partials (p4 0..2) start as soon as the attention PSUM drains;
            # the p4=3 matmuls accumulate into the same banks once ot3 is
            # normalized; one bias-add per it-tile evicts straight to bf16
            fpt = [None] * 8
            wave1 = [0, 1, 2, 3, 4, 5]
            for it in wave1:
                pool = psB if it < 4 else psS
                fpt[it] = pool.tile(
                    [128, 512], f32, tag=("B" if it < 4 else "S"), name=f"fp{it}"
                )
                for p4 in range(3):
                    nc.tensor.matmul(
                        fpt[it][:],
                        ot_tiles[p4][:, it * 128 : (it + 1) * 128],
                        wo[p4][:],
                        start=(p4 == 0),
                        stop=False,
                    )
            for it in wave1:
                nc.tensor.matmul(
                    fpt[it][:],
                    ot3[:, it * 128 : (it + 1) * 128],
                    wo[3][:],
                    start=False,
                    stop=True,
                )
            for it in wave1:
                os_t = obp.tile([128, DIM], bf16, tag="os", name=f"os{it}")
                nc.vector.tensor_add(os_t[:], fpt[it][:], bb_t[:])
                eng(it).dma_start(
                    out=out_d[it * 128 : (it + 1) * 128, :], in_=os_t[:]
                )
            for it in (6, 7):
                fpt[it] = psB.tile([128, 512], f32, tag="B", name=f"fp{it}")
                for p4 in range(4):
                    nc.tensor.matmul(
                        fpt[it][:],
                        ot_tiles[p4][:, it * 128 : (it + 1) * 128],
                        wo[p4][:],
                        start=(p4 == 0),
                        stop=(p4 == 3),
                    )
                os_t = obp.tile([128, DIM], bf16, tag="os", name=f"os{it}")
                nc.vector.tensor_add(os_t[:], fpt[it][:], bb_t[:])
                eng(it).dma_start(
                    out=out_d[it * 128 : (it + 1) * 128, :], in_=os_t[:]
                )

    _split_excess_waits(nc, mybir, bass_rust)
    _CACHE["nc"] = nc
    return nc


def _prep_inputs(inputs):
    import ml_dtypes

    bfnp = ml_dtypes.bfloat16
    x = np.ascontiguousarray(inputs["x"], dtype=np.float32)
    w_qkv = np.ascontiguousarray(inputs["w_qkv"], dtype=np.float32)
    w_out = np.ascontiguousarray(inputs["w_out"], dtype=np.float32)
    b_out = np.ascontiguousarray(inputs["b_out"], dtype=np.float32)

    # per-head slices of the fused qkv weight
    wq = [w_qkv[:, h * 192 : h * 192 + 64] for h in range(HEAD)]
    wk = [w_qkv[:, h * 192 + 64 : h * 192 + 128] for h in range(HEAD)]
    wv = [w_qkv[:, h * 192 + 128 : h * 192 + 192] for h in range(HEAD)]
    # pair-banded column order: [q0 q1 k0 k1 | q2 q3 k2 k3 | ...]
    blocks = []
    for p in range(4):
        blocks += [wq[2 * p], wq[2 * p + 1], wk[2 * p], wk[2 * p + 1]]
    w_qk = np.ascontiguousarray(np.concatenate(blocks, axis=1)).astype(bfnp)
    w_v = np.ascontiguousarray(np.concatenate(wv, axis=1)).astype(bfnp)

    in_maps = []
    for i in range(N_CORES):
        xT = np.ascontiguousarray(x[i].reshape(N, C).T).astype(bfnp)
        in_maps.append(
            {"xT": xT, "w_qk": w_qk, "w_v": w_v, "w_out": w_out, "b_out": b_out}
        )
    return in_maps


def _run(inputs, trace=False):
    _ensure_paths()
    import os

    if trace:
        os.environ.pop("BASS_NEVER_TRACE", None)
    else:
        os.environ["BASS_NEVER_TRACE"] = "1"
    from concourse import bass_utils

    nc = _build()
    in_maps = _prep_inputs(inputs)
    res = bass_utils.run_bass_kernel_spmd(
        nc, in_maps, core_ids=list(range(N_CORES)), trace=trace
    )
    out = np.stack(
        [res.results[i]["out"].reshape(32, 32, DIM) for i in range(N_CORES)]
    ).astype(np.float32)
    return out, res


def kernel(**inputs):
    out, _ = _run(inputs, trace=False)
    return out


# revision 3
# speedup vs baseline: 1.0230x; 1.0230x over previous
"""Trainium2 Bass kernel for nn_Attention_85796266705382 (v3).

Reference computation (per batch element, b=8, HEAD=8, n=32*32=1024, c=dim=512):
    qkv = x @ w_qkv                      # (n, 1536), per-head interleaved [q|k|v] x 64
    q,k,v per head (n, 64)
    attn = softmax(q @ k.T * 8**-0.5)    # scale uses FULL batch size (reference quirk)
    out  = attn @ v                      # (n, 64) per head -> (n, 512)
    y    = out @ w_out + b_out           # (n, 512)

Sharding: pure data-parallel over batch - one batch element per NeuronCore.

v3 design:
  * scores: zero-padded 128-contraction (uniform PE tile mode, proven 216ns
    steady-state, no LDW-transition exposure).
  * AV results are EVICTED from PSUM to SBUF (f32r) right after accumulation:
    PSUM banks free deterministically (no DMA in any PE-gating path), so the
    next pair's AV matmuls interleave into the exp-gated scores phase instead
    of serializing as a burst at round end.
  * den path: the evicted row 64 (ones-column sums) is DMA-gathered to
    [128,16], reciprocal'd in ~260ns (instead of the 3us/pair transpose
    dance), bounced via DRAM, and broadcast with a stride-0 partition AP.
    Normalizing mults read only SBUF and gate nothing on the PE side.
  * input loads alternate the Sync/Scalar HWDGE rings to halve the ~600ns
    per-dma issue serialization in the preamble.
  * tail: ysub partials overlap pair-3's den DMA chain; no one-hot broadcast
    matmuls.
"""

import numpy as np


def _ensure_paths():
    import sys

    try:
        import concourse.bass  # noqa: F401

        return
    except ImportError:
        pass
    for p in ("/opt/trn_rl_repo", "/root/.axon_site/_ro/trn_rl_repo"):
        if p not in sys.path:
            sys.path.append(p)
    import concourse.bass  # noqa: F401


HEAD = 8
B = 8
N = 1024  # tokens per batch element (32*32)
C = 512  # channels
DIM = 512
DH = 64
SCALE = float(B) ** -0.5  # reference scales by batch size, reproduced faithfully
N_CORES = 8

_CACHE = {}


def _split_excess_waits(nc, mybir, bass_rust):
    """walrus in this container accepts 1 sync wait per instruction (2 for
    EventSemaphore); Tile sometimes attaches more. Hoist the excess onto fresh
    same-engine NoOps inserted just before the over-capacity instruction."""
    n_split = 0
    for fn in nc.m.functions:
        for bb in fn.blocks:
            insts = bb.instructions
            i = 0
            while i < len(insts):
                inst = insts[i]
                si = inst.sync_info
                cap = 2 if isinstance(inst, mybir.InstEventSemaphore) else 1
                if si is not None and len(si.on_wait) > cap:
                    extra = list(si.on_wait[cap:])
                    del si.on_wait[cap:]
                    new_insts = []
                    for k in range(0, len(extra), 2):
                        pair = extra[k : k + 2]
                        nop = mybir.InstEventSemaphore(
                            name=f"{inst.name}_ws{k}", ins=[], outs=[]
                        )
                        nop.engine = inst.engine
                        nop.sync_info = bass_rust.SyncInfo(on_wait=pair, on_update=[])
                        new_insts.append(nop)
                        n_split += 1
                    insts[i:i] = new_insts
                    i += len(new_insts)
                i += 1
    return n_split


def _build():
    if "nc" in _CACHE:
        return _CACHE["nc"]
    _ensure_paths()
    import bass_rust
    import concourse.bass as bass
    import concourse.mybir as mybir
    import concourse.tile as tile

    f32 = mybir.dt.float32
    f32r = mybir.dt.float32r
    bf16 = mybir.dt.bfloat16
    u16 = mybir.dt.uint16
    u32 = mybir.dt.uint32
    Exp = mybir.ActivationFunctionType.Exp
    Copy = mybir.ActivationFunctionType.Copy

    nc = bass.Bass(trn_type="TRN2", target_bir_lowering=False, debug=False)

    xT_d = nc.dram_tensor("xT", [C, N], bf16, kind="ExternalInput").ap()
    wqk_d = nc.dram_tensor("w_qk", [C, 2 * DIM], bf16, kind="ExternalInput").ap()
    wv_d = nc.dram_tensor("w_v", [C, DIM], bf16, kind="ExternalInput").ap()
    wo_d = nc.dram_tensor("w_out", [DIM, DIM], f32r, kind="ExternalInput").ap()
    b_d = nc.dram_tensor("b_out", [DIM], f32r, kind="ExternalInput").ap()
    out_d = nc.dram_tensor("out", [N, DIM], bf16, kind="ExternalOutput").ap()
    # per-pair reciprocal-den bounce slots (all distinct: no WAR hazards)
    rd2_d = nc.dram_tensor("rd2_scratch", [4, 2 * N], bf16).ap()

    with tile.TileContext(nc) as tc:
        with (
            tc.tile_pool(name="wp", bufs=1) as wp,
            tc.tile_pool(name="xp", bufs=1) as xp,
            tc.tile_pool(name="qkp", bufs=1) as qkp,
            tc.tile_pool(name="vpp", bufs=1) as vpp,
            tc.tile_pool(name="ptp", bufs=28) as ptp,
            tc.tile_pool(name="otp", bufs=1) as otp,
            tc.tile_pool(name="evp", bufs=4) as evp,
            tc.tile_pool(name="gp", bufs=3) as gp,
            tc.tile_pool(name="bcp", bufs=3) as bcp,
            tc.tile_pool(name="ysp", bufs=1) as ysp,
            tc.tile_pool(name="obp", bufs=3) as obp,
            tc.tile_pool(name="psS", bufs=2, space="PSUM") as psS,
            tc.tile_pool(name="psB", bufs=4, space="PSUM") as psB,
        ):
            # ---- input loads: first-needed chunks first, alternating the two
            # HWDGE rings (sync / scalar) to parallelize descriptor issue ----
            def eng(i):
                return nc.sync if i % 2 == 0 else nc.scalar

            xT, wqk = [], []
            for ct in range(4):
                xT.append(xp.tile([128, N], bf16, tag=f"xT{ct}", name=f"xT{ct}"))
                wqk.append(
                    wp.tile([128, 2 * DIM], bf16, tag=f"wqk{ct}", name=f"wqk{ct}")
                )
            for ct in range(4):
                eng(ct).dma_start(
                    out=wqk[ct][:, 0:256], in_=wqk_d[ct * 128 : (ct + 1) * 128, 0:256]
                )
                eng(ct + 1).dma_start(
                    out=xT[ct][:], in_=xT_d[ct * 128 : (ct + 1) * 128, :]
                )
            for ct in range(4):
                eng(ct + 1).dma_start(
                    out=wqk[ct][:, 256:1024],
                    in_=wqk_d[ct * 128 : (ct + 1) * 128, 256:1024],
                )
            wv = []
            for ct in range(4):
                t = wp.tile([128, DIM], bf16, tag=f"wv{ct}", name=f"wv{ct}")
                eng(ct).dma_start(out=t[:], in_=wv_d[ct * 128 : (ct + 1) * 128, :])
                wv.append(t)

            def xTs(ct, a, b):
                return xT[ct][:, a:b]

            def wqks(ct, a, b):
                return wqk[ct][:, a:b]

            # zero-padded K-tiles: kp[par][w] holds head-w's K rows in the
            # same 64-row band as the Q tile layout, other 64 rows ZERO, so
            # score matmuls contract over the full 128 partitions (exact).
            kp = {}
            for par in range(2):
                for w in range(2):
                    t = qkp.tile([128, N], bf16, tag=f"kp{par}{w}", name=f"kp{par}{w}")
                    if w == 0:
                        nc.vector.memset(t[64:128, :].bitcast(u16), 0)
                    else:
                        nc.vector.memset(t[0:64, :].bitcast(u16), 0)
                    kp[(par, w)] = t

            def qk_tile(dt_i):
                """one d-tile of the transposed qk projection -> SBUF bf16.
                Odd d-tiles (K) are written as the zero-padded kp pair."""
                ps = psS.tile([128, N], f32, tag="S", name=f"qkps{dt_i}")
                for ch in range(2):
                    for ct in range(4):
                        nc.tensor.matmul(
                            ps[:, ch * 512 : (ch + 1) * 512],
                            wqks(ct, dt_i * 128, (dt_i + 1) * 128),
                            xTs(ct, ch * 512, (ch + 1) * 512),
                            start=(ct == 0),
                            stop=(ct == 3),
                        )
                if dt_i % 2 == 0:
                    t = qkp.tile([128, N], bf16, tag=f"qk{dt_i}", name=f"qk{dt_i}")
                    nc.vector.tensor_copy(t[:], ps[:])
                    return t
                par = (dt_i // 2) % 2
                nc.vector.tensor_copy(kp[(par, 0)][0:64, :], ps[0:64, :])
                nc.vector.tensor_copy(kp[(par, 1)][64:128, :], ps[64:128, :])
                return par

            def scores_jt(p, jt, QT, par):
                """score matmuls (zero-padded K=128) + exp for one (pair, jt)"""
                sps = [
                    psS.tile([128, N], f32, tag="S", name=f"s_{p}_{jt}_{w}")
                    for w in range(2)
                ]
                for w in range(2):
                    for ch in range(2):
                        nc.tensor.matmul(
                            sps[w][:, ch * 512 : (ch + 1) * 512],
                            kp[(par, w)][:, jt * 128 : (jt + 1) * 128],
                            QT[:, ch * 512 : (ch + 1) * 512],
                            start=True,
                            stop=True,
                        )
                pts = []
                for w in range(2):
                    pt = ptp.tile([128, N], bf16, tag="pt", name=f"pt{p}_{jt}_{w}")
                    nc.scalar.activation(pt[:], sps[w][:], Exp, scale=SCALE)
                    pts.append(pt)
                return pts

            def v_jt(jt):
                """one j-tile of the v projection, 65-pitch + ones column"""
                vt = vpp.tile([128, HEAD, DH + 1], bf16, tag=f"v{jt}", name=f"v{jt}")
                nc.vector.memset(vt[:, :, DH : DH + 1].bitcast(u16), 16256)
                ps = psB.tile([128, 512], f32, tag="B", name=f"vps{jt}")
                for ct in range(4):
                    nc.tensor.matmul(
                        ps[:],
                        xTs(ct, jt * 128, (jt + 1) * 128),
                        wv[ct][:],
                        start=(ct == 0),
                        stop=(ct == 3),
                    )
                nc.vector.tensor_copy(
                    vt[:, :, 0:DH],
                    ps[:].rearrange("p (h e) -> p h e", h=HEAD),
                )
                return vt

            def make_av(p):
                return [
                    [
                        psB.tile([128, 512], f32, tag="B", name=f"av{p}_{w}_{c}")
                        for c in range(2)
                    ]
                    for w in range(2)
                ]

            def av_mm(p, jtAV, w, ch, av, pts):
                nc.tensor.matmul(
                    av[w][ch][0 : DH + 1, :],
                    v_sb[jtAV][:, 2 * p + w, :],
                    pts[jtAV][w][:, ch * 512 : (ch + 1) * 512],
                    start=(jtAV == 0),
                    stop=(jtAV == 7),
                )

            # flattened AV matmul order for one pair (32 mms)
            AV_ORDER = [
                (jtAV, w, ch) for jtAV in range(8) for w in range(2) for ch in range(2)
            ]
            # spread over scores-jt 0..5 so evict/den complete mid-round
            AV_SPLITS = [0, 6, 12, 18, 24, 28, 32]

            def evict(p, av):
                """PSUM -> SBUF eviction of the AV result (+ den row 64).
                Frees the PSUM banks deterministically so the next pair's AV
                can interleave; nothing downstream gates the PE. One [65, 2N]
                tile per pair so the den row is a single contiguous [1, 2N]."""
                ev = evp.tile([DH + 1, 2 * N], f32r, tag="ev", name=f"ev{p}")
                for w in range(2):
                    for ch in range(2):
                        nc.vector.tensor_copy(
                            ev[:, w * N + ch * 512 : w * N + (ch + 1) * 512],
                            av[w][ch][0 : DH + 1, :],
                        )
                return ev

            def den_chain(p, ev):
                """den row -> 1/den broadcast tile: one DMA gather to [128,16],
                tiny reciprocal, one DMA back to DRAM, one stride-0 partition
                broadcast (4D AP deinterleaves the w halves)."""
                gi = gp.tile([128, 16], f32r, tag="gi", name=f"gi{p}")
                nc.sync.dma_start(out=gi[:], in_=ev[64:65, :])
                go = gp.tile([128, 16], bf16, tag="go", name=f"go{p}")
                with nc.allow_low_precision(reason="1/den tolerates bf16"):
                    nc.vector.reciprocal(go[:], gi[:])
                nc.sync.dma_start(out=rd2_d[p : p + 1, :], in_=go[:])
                bc = bcp.tile([64, 2, N], bf16, tag="bc", name=f"bc{p}")
                # flat gather/back round-trip keeps the two w halves contiguous
                bc_src = bass.AP(
                    tensor=rd2_d.tensor, offset=p * 2 * N, ap=[[0, 64], [1, 2 * N]]
                )
                nc.sync.dma_start(out=bc[:], in_=bc_src)
                return bc

            def mults(p, ev, bc, ot):
                """normalizing multiplies, pure SBUF (gates nothing on PE)"""
                for w in range(2):
                    nc.vector.tensor_mul(
                        ot[w * 64 : (w + 1) * 64, :],
                        ev[0:DH, w * N : (w + 1) * N],
                        bc[:, w, :],
                    )

            # ================= round 0: qk pair 0 + scores p0 + v proj =======
            qk = {}
            qk[0] = qk_tile(0)
            par0 = qk_tile(1)

            v_sb = [None] * 8
            pts_prev = [None] * 8
            for jt in range(8):
                pts_prev[jt] = scores_jt(0, jt, qk[0], par0)
                v_sb[jt] = v_jt(jt)
                if jt == 4:
                    qk[2] = qk_tile(2)
                if jt == 6:
                    qk[3] = qk_tile(3)

            ot_tiles = [None] * 4
            av3 = None
            evs3 = None

            # ================= rounds 1-3: scores p | AV p-1 =================
            # AV for pair p-1 is front-loaded over jt 0..5 so its eviction +
            # den DMA chain complete mid-round and the normalizing mults land
            # at round end. Pair 3's own AV additionally starts inside round 3
            # (as its pts stream in) so the tail isn't gated on a full AV pass.
            for p in range(1, 4):
                QT, par = qk[2 * p], qk[2 * p + 1]
                av = make_av(p - 1)
                pts_cur = [None] * 8
                evs = None
                for jt in range(8):
                    pts_cur[jt] = scores_jt(p, jt, QT, par)
                    if jt < 6:
                        for jtAV, w, ch in AV_ORDER[AV_SPLITS[jt] : AV_SPLITS[jt + 1]]:
                            av_mm(p - 1, jtAV, w, ch, av, pts_prev)
                    elif p == 3:
                        # pair-3 AV rides the last exp-gated slots of round 3
                        av3 = av3 if av3 is not None else make_av(3)
                        lo, hi = (0, 2) if jt == 6 else (2, 5)
                        for jtAV in range(lo, hi):
                            for w in range(2):
                                for ch in range(2):
                                    av_mm(3, jtAV, w, ch, av3, pts_cur)
                    if jt == 5:
                        evs = evict(p - 1, av)
                        bc = den_chain(p - 1, evs)
                        if p == 3:
                            # DVE is otherwise idle until evict(3); emitting
                            # the ot2 mults here lets ysub start at round end
                            ot_tiles[2] = otp.tile(
                                [128, N], f32r, tag="ot2", name="ot2"
                            )
                            mults(2, evs, bc, ot_tiles[2])
                    if jt == 4:
                        if p < 3:
                            qk[2 * p + 2] = qk_tile(2 * p + 2)
                        else:
                            # late weight loads (final projection only); kept
                            # off the scalar ring so the exp stream stays fed
                            wo = []
                            for p4 in range(4):
                                t = wp.tile(
                                    [128, DIM], f32r, tag=f"wo{p4}", name=f"wo{p4}"
                                )
                                nc.sync.dma_start(
                                    out=t[:], in_=wo_d[p4 * 128 : (p4 + 1) * 128, :]
                                )
                                wo.append(t)
                            bb_t = wp.tile([128, DIM], f32r, tag="bb", name="bb")
                            b_src = bass.AP(
                                tensor=b_d.tensor,
                                offset=b_d.offset,
                                ap=[[0, 128]] + list(b_d.ap),
                            )
                            nc.sync.dma_start(out=bb_t[:], in_=b_src)
                    if jt == 6 and p < 3:
                        qk[2 * p + 3] = qk_tile(2 * p + 3)
                if p < 3:
                    ot_tiles[p - 1] = otp.tile(
                        [128, N], f32r, tag=f"ot{p - 1}", name=f"ot{p - 1}"
                    )
                    mults(p - 1, evs, bc, ot_tiles[p - 1])
                pts_prev = pts_cur

            # ================= round 4: finish AV p3 | ysub | tail ===========
            for jtAV in range(5, 8):
                for w in range(2):
                    for ch in range(2):
                        av_mm(3, jtAV, w, ch, av3, pts_prev)
            evs3 = evict(3, av3)
            bc3 = den_chain(3, evs3)
            ot3 = otp.tile([128, N], f32r, tag="ot3", name="ot3")
            ot_tiles[3] = ot3
            mults(3, evs3, bc3, ot3)

            # ========= tail: fused final projection (waves over PSUM) ======
            # partials (p4 0..2) start as soon as the attention PSUM drains;
            # the p4=3 matmuls accumulate into the same banks once ot3 is
            # normalized; one bias-add per it-tile evicts straight to bf16
            fpt = [None] * 8
            wave1 = [0, 1, 2, 3, 4, 5]
            for it in wave1:
                pool = psB if it < 4 else psS
                fpt[it] = pool.tile(
                    [128, 512], f32, tag=("B" if it < 4 else "S"), name=f"fp{it}"
                )
                for p4 in range(3):
                    nc.tensor.matmul(
                        fpt[it][:],
                        ot_tiles[p4][:, it * 128 : (it + 1) * 128],
                        wo[p4][:],
                        start=(p4 == 0),
                        stop=False,
                    )
            for it in wave1:
                nc.tensor.matmul(
                    fpt[it][:],
                    ot3[:, it * 128 : (it + 1) * 128],
                    wo[3][:],
                    start=False,
                    stop=True,
                )
            for it in wave1:
                os_t = obp.tile([128, DIM], bf16, tag="os", name=f"os{it}")
                nc.vector.tensor_add(os_t[:], fpt[it][:], bb_t[:])
                eng(it).dma_start(
                    out=out_d[it * 128 : (it + 1) * 128, :], in_=os_t[:]
                )
            for it in (6, 7):
                fpt[it] = psB.tile([128, 512], f32, tag="B", name=f"fp{it}")
                for p4 in range(4):
                    nc.tensor.matmul(
                        fpt[it][:],
                        ot_tiles[p4][:, it * 128 : (it + 1) * 128],
                        wo[p4][:],
                        start=(p4 == 0),
                        stop=(p4 == 3),
                    )
                os_t = obp.tile([128, DIM], bf16, tag="os", name=f"os{it}")
                nc.vector.tensor_add(os_t[:], fpt[it][:], bb_t[:])
                eng(it).dma_start(
                    out=out_d[it * 128 : (it + 1) * 128, :], in_=os_t[:]
                )

    _split_excess_waits(nc, mybir, bass_rust)
    _CACHE["nc"] = nc
    return nc


def _prep_inputs(inputs):
    import ml_dtypes

    bfnp = ml_dtypes.bfloat16
    x = np.ascontiguousarray(inputs["x"], dtype=np.float32)
    w_qkv = np.ascontiguousarray(inputs["w_qkv"], dtype=np.float32)
    w_out = np.ascontiguousarray(inputs["w_out"], dtype=np.float32)
    b_out = np.ascontiguousarray(inputs["b_out"], dtype=np.float32)

    # per-head slices of the fused qkv weight
    wq = [w_qkv[:, h * 192 : h * 192 + 64] for h in range(HEAD)]
    wk = [w_qkv[:, h * 192 + 64 : h * 192 + 128] for h in range(HEAD)]
    wv = [w_qkv[:, h * 192 + 128 : h * 192 + 192] for h in range(HEAD)]
    # pair-banded column order: [q0 q1 k0 k1 | q2 q3 k2 k3 | ...]
    blocks = []
    for p in range(4):
        blocks += [wq[2 * p], wq[2 * p + 1], wk[2 * p], wk[2 * p + 1]]
    w_qk = np.ascontiguousarray(np.concatenate(blocks, axis=1)).astype(bfnp)
    w_v = np.ascontiguousarray(np.concatenate(wv, axis=1)).astype(bfnp)

    in_maps = []
    for i in range(N_CORES):
        xT = np.ascontiguousarray(x[i].reshape(N, C).T).astype(bfnp)
        in_maps.append(
            {"xT": xT, "w_qk": w_qk, "w_v": w_v, "w_out": w_out, "b_out": b_out}
        )
    return in_maps


def _run(inputs, trace=False):
    _ensure_paths()
    import os

    if trace:
        os.environ.pop("BASS_NEVER_TRACE", None)
    else:
        os.environ["BASS_NEVER_TRACE"] = "1"
    from concourse import bass_utils

    nc = _build()
    in_maps = _prep_inputs(inputs)
    res = bass_utils.run_bass_kernel_spmd(
        nc, in_maps, core_ids=list(range(N_CORES)), trace=trace
    )
    out = np.stack(
        [res.results[i]["out"].reshape(32, 32, DIM) for i in range(N_CORES)]
    ).astype(np.float32)
    return out, res


def kernel(**inputs):
    out, _ = _run(inputs, trace=False)
    return out


# revision 6
# speedup vs baseline: 1.0432x; 1.0198x over previous
"""Trainium2 Bass kernel for nn_Attention_85796266705382.

Per batch element (b=8, HEAD=8, n=1024, c=dim=512), one element per core:
    qkv = x @ w_qkv; per-head q,k,v (n, 64)
    attn = softmax(q @ k.T * 8**-0.5)   # scale uses batch size (ref quirk)
    y    = (attn @ v) @ w_out + b_out

Design (~123us HW; session start was ~145us):
  * steady state is exp-bound: ScalarE runs 64 x [128,1024] Exp nearly
    back-to-back (~66us floor); each round scores pair p while pair p-1's
    AV matmuls fill the PE idle slack between exp-gated score groups.
  * scores: zero-padded 128-contraction, uniform PE tile mode (216ns/MM
    steady). 64-row dual-tile scores measured WORSE (PSUM-slot semaphores
    stagger the pair, no concurrency, partial HAM cooldown).
  * AV is EVICTED from PSUM to SBUF f32r [65, 2N] right after accumulation:
    banks free deterministically (no DMA in any PE-gating path) so the next
    pair's AV interleaves instead of bursting; AV is front-loaded over
    scores-jt 0..5 (AV_SPLITS) so the den chain completes mid-round.
  * den path: evicted ones-column row -> SBUF->SBUF DMA gather to [128,16]
    -> ~260ns reciprocal (replaces a 3us/pair transpose dance) -> bf16 DRAM
    bounce -> stride-0 partition-broadcast [64,2,N]; the normalizing mults
    are pure-SBUF DVE ops, nothing on the PE side waits for DMA.
  * tail: fused final projection, 4-matmul PSUM accumulation per it-tile
    waved over psB+psS banks; p4=0..2 partials run under pair-3's den
    chain; one bias-add per tile evicts straight to bf16 output (halves
    the store drain; ~0.2% extra rounding vs the 2e-2 budget).
  * preamble: input loads alternate the sync/scalar HWDGE rings; 12 dummy
    matmuls + a dummy Exp during the DMA wait pre-warm the HAM clock gate
    and hoist the ~2.7us exp-table load off the critical path.
  * WARNINGS: (1) Tile's schedule is sensitive to SOURCE LINE NUMBERS -
    any edit that shifts code lines re-rolls the schedule (measured
    123->149us on a docstring-only edit; pads were searched to pick this
    schedule). Keep edits line-count-neutral and re-measure.
    (2) Do not co-schedule qk-prefetch chunks densely into the exp-gated
    slots: with all engines saturated EVERY engine slows ~15% uniformly
    (SBUF/PSUM-port or power ceiling; 126->146us, reproducible). The two
    ~1.7us exp gaps per round are load-bearing slack.







"""

import numpy as np


def _ensure_paths():
    import sys

    try:
        import concourse.bass  # noqa: F401

        return
    except ImportError:
        pass
    for p in ("/opt/trn_rl_repo", "/root/.axon_site/_ro/trn_rl_repo"):
        if p not in sys.path:
            sys.path.append(p)
    import concourse.bass  # noqa: F401


HEAD = 8
B = 8
N = 1024  # tokens per batch element (32*32)
C = 512  # channels
DIM = 512
DH = 64
SCALE = float(B) ** -0.5  # reference scales by batch size, reproduced faithfully
N_CORES = 8

_CACHE = {}


def _split_excess_waits(nc, mybir, bass_rust):
    """walrus in this container accepts 1 sync wait per instruction (2 for
    EventSemaphore); Tile sometimes attaches more. Hoist the excess onto fresh
    same-engine NoOps inserted just before the over-capacity instruction."""
    n_split = 0
    for fn in nc.m.functions:
        for bb in fn.blocks:
            insts = bb.instructions
            i = 0
            while i < len(insts):
                inst = insts[i]
                si = inst.sync_info
                cap = 2 if isinstance(inst, mybir.InstEventSemaphore) else 1
                if si is not None and len(si.on_wait) > cap:
                    extra = list(si.on_wait[cap:])
                    del si.on_wait[cap:]
                    new_insts = []
                    for k in range(0, len(extra), 2):
                        pair = extra[k : k + 2]
                        nop = mybir.InstEventSemaphore(
                            name=f"{inst.name}_ws{k}", ins=[], outs=[]
                        )
                        nop.engine = inst.engine
                        nop.sync_info = bass_rust.SyncInfo(on_wait=pair, on_update=[])
                        new_insts.append(nop)
                        n_split += 1
                    insts[i:i] = new_insts
                    i += len(new_insts)
                i += 1
    return n_split


def _build():
    if "nc" in _CACHE:
        return _CACHE["nc"]
    _ensure_paths()
    import bass_rust
    import concourse.bass as bass
    import concourse.mybir as mybir
    import concourse.tile as tile

    f32 = mybir.dt.float32
    f32r = mybir.dt.float32r
    bf16 = mybir.dt.bfloat16
    u16 = mybir.dt.uint16
    u32 = mybir.dt.uint32
    Exp = mybir.ActivationFunctionType.Exp
    Copy = mybir.ActivationFunctionType.Copy

    nc = bass.Bass(trn_type="TRN2", target_bir_lowering=False, debug=False)

    xT_d = nc.dram_tensor("xT", [C, N], bf16, kind="ExternalInput").ap()
    wqk_d = nc.dram_tensor("w_qk", [C, 2 * DIM], bf16, kind="ExternalInput").ap()
    wv_d = nc.dram_tensor("w_v", [C, DIM], bf16, kind="ExternalInput").ap()
    wo_d = nc.dram_tensor("w_out", [DIM, DIM], f32r, kind="ExternalInput").ap()
    b_d = nc.dram_tensor("b_out", [DIM], f32r, kind="ExternalInput").ap()
    out_d = nc.dram_tensor("out", [N, DIM], bf16, kind="ExternalOutput").ap()
    # per-pair reciprocal-den bounce slots (all distinct: no WAR hazards)
    rd2_d = nc.dram_tensor("rd2_scratch", [4, 2 * N], bf16).ap()

    with tile.TileContext(nc) as tc:
        with (
            tc.tile_pool(name="wp", bufs=1) as wp,
            tc.tile_pool(name="xp", bufs=1) as xp,
            tc.tile_pool(name="qkp", bufs=1) as qkp,
            tc.tile_pool(name="vpp", bufs=1) as vpp,
            tc.tile_pool(name="ptp", bufs=28) as ptp,
            tc.tile_pool(name="otp", bufs=1) as otp,
            tc.tile_pool(name="evp", bufs=4) as evp,
            tc.tile_pool(name="gp", bufs=3) as gp,
            tc.tile_pool(name="bcp", bufs=3) as bcp,
            tc.tile_pool(name="rdp", bufs=1) as rdp,
            tc.tile_pool(name="obp", bufs=3) as obp,
            tc.tile_pool(name="psS", bufs=2, space="PSUM") as psS,
            tc.tile_pool(name="psB", bufs=4, space="PSUM") as psB,
        ):
            # ---- input loads: first-needed chunks first, alternating the two
            # HWDGE rings (sync / scalar) to parallelize descriptor issue ----
            def eng(i):
                return nc.sync if i % 2 == 0 else nc.scalar

            xT, wqk = [], []
            for ct in range(4):
                xT.append(xp.tile([128, N], bf16, tag=f"xT{ct}", name=f"xT{ct}"))
                wqk.append(
                    wp.tile([128, 2 * DIM], bf16, tag=f"wqk{ct}", name=f"wqk{ct}")
                )
            for ct in range(4):
                eng(ct).dma_start(
                    out=wqk[ct][:, 0:256], in_=wqk_d[ct * 128 : (ct + 1) * 128, 0:256]
                )
                eng(ct + 1).dma_start(
                    out=xT[ct][:], in_=xT_d[ct * 128 : (ct + 1) * 128, :]
                )
            for ct in range(4):
                eng(ct + 1).dma_start(
                    out=wqk[ct][:, 256:1024],
                    in_=wqk_d[ct * 128 : (ct + 1) * 128, 256:1024],
                )
            wv = []
            for ct in range(4):
                t = wp.tile([128, DIM], bf16, tag=f"wv{ct}", name=f"wv{ct}")
                eng(ct).dma_start(out=t[:], in_=wv_d[ct * 128 : (ct + 1) * 128, :])
                wv.append(t)

            def xTs(ct, a, b):
                return xT[ct][:, a:b]

            def wqks(ct, a, b):
                return wqk[ct][:, a:b]

            # PE warm-up + ACT-table preload during the DMA preamble: the
            # HAM clock gate needs ~3.4us of sustained matmul activity before
            # the PE runs at 2.4GHz, and the exp table load costs ~2.7us on
            # first use; both are paid here while the PE/ACT queues are idle
            # waiting for the input DMAs.
            wu = wp.tile([128, 512], bf16, tag="wu", name="wu")
            nc.vector.memset(wu[:].bitcast(u16), 0)
            wups = psB.tile([128, 512], f32, tag="B", name="wups")
            for wi in range(12):
                nc.tensor.matmul(
                    wups[:], wu[:, 0:128], wu[:], start=True, stop=True
                )
            wacto = wp.tile([1, 8], bf16, tag="wacto", name="wacto")
            nc.scalar.activation(wacto[:], wu[0:1, 0:8], Exp, scale=1.0)

            # one-hot stationary + zeroed row-broadcast landing tile for the
            # pair-3 den path (PE broadcast instead of DRAM bounce)
            ones_mm = wp.tile([128, 128], f32r, tag="ones_mm", name="ones_mm")
            nc.vector.memset(ones_mm[:].bitcast(mybir.dt.uint32), 0)
            nc.vector.memset(
                ones_mm[0:1, :].bitcast(mybir.dt.uint32), 1065353216
            )
            rdr = rdp.tile([128, 2 * N], f32r, tag="rdr", name="rdr")
            nc.vector.memset(rdr[:].bitcast(mybir.dt.uint32), 0)

            # zero-padded K-tiles: kp[par][w] holds head-w's K rows in the
            # same 64-row band as the Q tile layout, other 64 rows ZERO, so
            # score matmuls contract over the full 128 partitions (exact).
            kp = {}
            for par in range(2):
                for w in range(2):
                    t = qkp.tile([128, N], bf16, tag=f"kp{par}{w}", name=f"kp{par}{w}")
                    if w == 0:
                        nc.vector.memset(t[64:128, :].bitcast(u16), 0)
                    else:
                        nc.vector.memset(t[0:64, :].bitcast(u16), 0)
                    kp[(par, w)] = t

            def qk_tile(dt_i):
                """one d-tile of the transposed qk projection -> SBUF bf16.
                Odd d-tiles (K) are written as the zero-padded kp pair."""
                ps = psS.tile([128, N], f32, tag="S", name=f"qkps{dt_i}")
                for ch in range(2):
                    for ct in range(4):
                        nc.tensor.matmul(
                            ps[:, ch * 512 : (ch + 1) * 512],
                            wqks(ct, dt_i * 128, (dt_i + 1) * 128),
                            xTs(ct, ch * 512, (ch + 1) * 512),
                            start=(ct == 0),
                            stop=(ct == 3),
                        )
                if dt_i % 2 == 0:
                    t = qkp.tile([128, N], bf16, tag=f"qk{dt_i}", name=f"qk{dt_i}")
                    nc.vector.tensor_copy(t[:], ps[:])
                    return t
                par = (dt_i // 2) % 2
                nc.vector.tensor_copy(kp[(par, 0)][0:64, :], ps[0:64, :])
                nc.vector.tensor_copy(kp[(par, 1)][64:128, :], ps[64:128, :])
                return par

            def scores_jt(p, jt, QT, par):
                """score matmuls (zero-padded K=128) + exp for one (pair, jt)"""
                sps = [
                    psS.tile([128, N], f32, tag="S", name=f"s_{p}_{jt}_{w}")
                    for w in range(2)
                ]
                for w in range(2):
                    for ch in range(2):
                        nc.tensor.matmul(
                            sps[w][:, ch * 512 : (ch + 1) * 512],
                            kp[(par, w)][:, jt * 128 : (jt + 1) * 128],
                            QT[:, ch * 512 : (ch + 1) * 512],
                            start=True,
                            stop=True,
                        )
                pts = []
                for w in range(2):
                    pt = ptp.tile([128, N], bf16, tag="pt", name=f"pt{p}_{jt}_{w}")
                    nc.scalar.activation(pt[:], sps[w][:], Exp, scale=SCALE)
                    pts.append(pt)
                return pts

            def v_jt(jt):
                """one j-tile of the v projection, 65-pitch + ones column"""
                vt = vpp.tile([128, HEAD, DH + 1], bf16, tag=f"v{jt}", name=f"v{jt}")
                nc.vector.memset(vt[:, :, DH : DH + 1].bitcast(u16), 16256)
                ps = psB.tile([128, 512], f32, tag="B", name=f"vps{jt}")
                for ct in range(4):
                    nc.tensor.matmul(
                        ps[:],
                        xTs(ct, jt * 128, (jt + 1) * 128),
                        wv[ct][:],
                        start=(ct == 0),
                        stop=(ct == 3),
                    )
                nc.vector.tensor_copy(
                    vt[:, :, 0:DH],
                    ps[:].rearrange("p (h e) -> p h e", h=HEAD),
                )
                return vt

            def make_av(p):
                return [
                    [
                        psB.tile([128, 512], f32, tag="B", name=f"av{p}_{w}_{c}")
                        for c in range(2)
                    ]
                    for w in range(2)
                ]

            def av_mm(p, jtAV, w, ch, av, pts):
                nc.tensor.matmul(
                    av[w][ch][0 : DH + 1, :],
                    v_sb[jtAV][:, 2 * p + w, :],
                    pts[jtAV][w][:, ch * 512 : (ch + 1) * 512],
                    start=(jtAV == 0),
                    stop=(jtAV == 7),
                )

            # flattened AV matmul order for one pair (32 mms)
            AV_ORDER = [
                (jtAV, w, ch) for jtAV in range(8) for w in range(2) for ch in range(2)
            ]
            # spread over scores-jt 0..5 so evict/den complete mid-round
            AV_SPLITS = [0, 6, 12, 18, 24, 28, 32]

            def evict(p, av):
                """PSUM -> SBUF eviction of the AV result (+ den row 64).
                Frees the PSUM banks deterministically so the next pair's AV
                can interleave; nothing downstream gates the PE. One [65, 2N]
                tile per pair so the den row is a single contiguous [1, 2N]."""
                ev = evp.tile([DH + 1, 2 * N], f32r, tag="ev", name=f"ev{p}")
                for w in range(2):
                    for ch in range(2):
                        nc.vector.tensor_copy(
                            ev[:, w * N + ch * 512 : w * N + (ch + 1) * 512],
                            av[w][ch][0 : DH + 1, :],
                        )
                return ev

            def den_chain(p, ev):
                """den row -> 1/den broadcast tile: one DMA gather to [128,16],
                tiny reciprocal, one DMA back to DRAM, one stride-0 partition
                broadcast (4D AP deinterleaves the w halves)."""
                gi = gp.tile([128, 16], f32r, tag="gi", name=f"gi{p}")
                nc.sync.dma_start(out=gi[:], in_=ev[64:65, :])
                go = gp.tile([128, 16], bf16, tag="go", name=f"go{p}")
                with nc.allow_low_precision(reason="1/den tolerates bf16"):
                    nc.vector.reciprocal(go[:], gi[:])
                nc.sync.dma_start(out=rd2_d[p : p + 1, :], in_=go[:])
                bc = bcp.tile([64, 2, N], bf16, tag="bc", name=f"bc{p}")
                # flat gather/back round-trip keeps the two w halves contiguous
                bc_src = bass.AP(
                    tensor=rd2_d.tensor, offset=p * 2 * N, ap=[[0, 64], [1, 2 * N]]
                )
                nc.sync.dma_start(out=bc[:], in_=bc_src)
                return bc

            def mults(p, ev, bc, ot):
                """normalizing multiplies, pure SBUF (gates nothing on PE)"""
                for w in range(2):
                    nc.vector.tensor_mul(
                        ot[w * 64 : (w + 1) * 64, :],
                        ev[0:DH, w * N : (w + 1) * N],
                        bc[:, w, :],
                    )

            # ================= round 0: qk pair 0 + scores p0 + v proj =======
            qk = {}
            qk[0] = qk_tile(0)
            par0 = qk_tile(1)

            v_sb = [None] * 8
            pts_prev = [None] * 8
            for jt in range(8):
                pts_prev[jt] = scores_jt(0, jt, qk[0], par0)
                v_sb[jt] = v_jt(jt)
                if jt == 4:
                    qk[2] = qk_tile(2)
                if jt == 6:
                    qk[3] = qk_tile(3)

            ot_tiles = [None] * 4
            av3 = None
            evs3 = None

            # ================= rounds 1-3: scores p | AV p-1 =================
            # AV for pair p-1 is front-loaded over jt 0..5 so its eviction +
            # den DMA chain complete mid-round and the normalizing mults land
            # at round end. Pair 3's own AV additionally starts inside round 3
            # (as its pts stream in) so the tail isn't gated on a full AV pass.
            for p in range(1, 4):
                QT, par = qk[2 * p], qk[2 * p + 1]
                av = make_av(p - 1)
                pts_cur = [None] * 8
                evs = None
                for jt in range(8):
                    pts_cur[jt] = scores_jt(p, jt, QT, par)
                    if jt < 6:
                        for jtAV, w, ch in AV_ORDER[AV_SPLITS[jt] : AV_SPLITS[jt + 1]]:
                            av_mm(p - 1, jtAV, w, ch, av, pts_prev)
                    elif p == 3:
                        # pair-3 AV rides the last exp-gated slots of round 3
                        av3 = av3 if av3 is not None else make_av(3)
                        lo, hi = (0, 2) if jt == 6 else (2, 5)
                        for jtAV in range(lo, hi):
                            for w in range(2):
                                for ch in range(2):
                                    av_mm(3, jtAV, w, ch, av3, pts_cur)
                    if jt == 5:
                        evs = evict(p - 1, av)
                        bc = den_chain(p - 1, evs)
                        if p == 3:
                            # DVE is otherwise idle until evict(3); emitting
                            # the ot2 mults here lets ysub start at round end
                            ot_tiles[2] = otp.tile(
                                [128, N], f32r, tag="ot2", name="ot2"
                            )
                            mults(2, evs, bc, ot_tiles[2])
                    if jt == 4:
                        if p < 3:
                            qk[2 * p + 2] = qk_tile(2 * p + 2)
                        else:
                            # late weight loads (final projection only); kept
                            # off the scalar ring so the exp stream stays fed
                            wo = []
                            for p4 in range(4):
                                t = wp.tile(
                                    [128, DIM], f32r, tag=f"wo{p4}", name=f"wo{p4}"
                                )
                                nc.sync.dma_start(
                                    out=t[:], in_=wo_d[p4 * 128 : (p4 + 1) * 128, :]
                                )
                                wo.append(t)
                            bb_t = wp.tile([128, DIM], f32r, tag="bb", name="bb")
                            b_src = bass.AP(
                                tensor=b_d.tensor,
                                offset=b_d.offset,
                                ap=[[0, 128]] + list(b_d.ap),
                            )
                            nc.sync.dma_start(out=bb_t[:], in_=b_src)
                    if jt == 6 and p < 3:
                        qk[2 * p + 3] = qk_tile(2 * p + 3)
                if p < 3:
                    ot_tiles[p - 1] = otp.tile(
                        [128, N], f32r, tag=f"ot{p - 1}", name=f"ot{p - 1}"
                    )
                    mults(p - 1, evs, bc, ot_tiles[p - 1])
                pts_prev = pts_cur

            # ================= round 4: finish AV p3 | ysub | tail ===========
            for jtAV in range(5, 8):
                for w in range(2):
                    for ch in range(2):
                        av_mm(3, jtAV, w, ch, av3, pts_prev)
            evs3 = evict(3, av3)
            # pair-3 den: gather -> tiny reciprocal -> SBUF row -> one-hot
            # matmul broadcast into PSUM (no DRAM bounce on the tail path)
            gi3 = gp.tile([128, 16], f32r, tag="gi", name="gi3")
            nc.sync.dma_start(out=gi3[:], in_=evs3[64:65, :])
            go3 = gp.tile([128, 16], f32r, tag="go", name="go3")
            with nc.allow_low_precision(reason="f32r == fp32 bits"):
                nc.vector.reciprocal(go3[:], gi3[:])
            nc.sync.dma_start(out=rdr[0:1, :], in_=go3[:])
            ot3 = otp.tile([128, N], f32r, tag="ot3", name="ot3")
            ot_tiles[3] = ot3

            # ========= tail: fused final projection (waves over PSUM) ======
            # wave-1 partials run during the den gather; the one-hot
            # broadcast + normalizing mults then unblock the p4=3 matmuls
            fpt = [None] * 8
            for it in range(4):
                fpt[it] = psB.tile([128, 512], f32, tag="B", name=f"fp{it}")
                for p4 in range(3):
                    nc.tensor.matmul(
                        fpt[it][:],
                        ot_tiles[p4][:, it * 128 : (it + 1) * 128],
                        wo[p4][:],
                        start=(p4 == 0),
                        stop=False,
                    )
            bc_ps = [
                psS.tile([128, N], f32, tag="S", name=f"bcps{w}") for w in range(2)
            ]
            for w in range(2):
                for ch in range(2):
                    nc.tensor.matmul(
                        bc_ps[w][:, ch * 512 : (ch + 1) * 512],
                        ones_mm[:],
                        rdr[:, w * N + ch * 512 : w * N + (ch + 1) * 512],
                        start=True,
                        stop=True,
                    )
            for w in range(2):
                nc.vector.tensor_mul(
                    ot3[w * 64 : (w + 1) * 64, :],
                    evs3[0:DH, w * N : (w + 1) * N],
                    bc_ps[w][0:64, :],
                )
            for it in range(4):
                nc.tensor.matmul(
                    fpt[it][:],
                    ot3[:, it * 128 : (it + 1) * 128],
                    wo[3][:],
                    start=False,
                    stop=True,
                )
                os_t = obp.tile([128, DIM], bf16, tag="os", name=f"os{it}")
                nc.vector.tensor_add(os_t[:], fpt[it][:], bb_t[:])
                eng(it).dma_start(
                    out=out_d[it * 128 : (it + 1) * 128, :], in_=os_t[:]
                )
            for it in (4, 5, 6, 7):
                fpt[it] = psB.tile([128, 512], f32, tag="B", name=f"fp{it}")
                for p4 in range(4):
                    nc.tensor.matmul(
                        fpt[it][:],
                        ot_tiles[p4][:, it * 128 : (it + 1) * 128],
                        wo[p4][:],
                        start=(p4 == 0),
                        stop=(p4 == 3),
                    )
                os_t = obp.tile([128, DIM], bf16, tag="os", name=f"os{it}")
                nc.vector.tensor_add(os_t[:], fpt[it][:], bb_t[:])
                eng(it).dma_start(
                    out=out_d[it * 128 : (it + 1) * 128, :], in_=os_t[:]
                )

    _split_excess_waits(nc, mybir, bass_rust)
    _CACHE["nc"] = nc
    return nc


def _prep_inputs(inputs):
    import ml_dtypes

    bfnp = ml_dtypes.bfloat16
    x = np.ascontiguousarray(inputs["x"], dtype=np.float32)
    w_qkv = np.ascontiguousarray(inputs["w_qkv"], dtype=np.float32)
    w_out = np.ascontiguousarray(inputs["w_out"], dtype=np.float32)
    b_out = np.ascontiguousarray(inputs["b_out"], dtype=np.float32)

    # per-head slices of the fused qkv weight
    wq = [w_qkv[:, h * 192 : h * 192 + 64] for h in range(HEAD)]
    wk = [w_qkv[:, h * 192 + 64 : h * 192 + 128] for h in range(HEAD)]
    wv = [w_qkv[:, h * 192 + 128 : h * 192 + 192] for h in range(HEAD)]
    # pair-banded column order: [q0 q1 k0 k1 | q2 q3 k2 k3 | ...]
    blocks = []
    for p in range(4):
        blocks += [wq[2 * p], wq[2 * p + 1], wk[2 * p], wk[2 * p + 1]]
    w_qk = np.ascontiguousarray(np.concatenate(blocks, axis=1)).astype(bfnp)
    w_v = np.ascontiguousarray(np.concatenate(wv, axis=1)).astype(bfnp)

    in_maps = []
    for i in range(N_CORES):
        xT = np.ascontiguousarray(x[i].reshape(N, C).T).astype(bfnp)
        in_maps.append(
            {"xT": xT, "w_qk": w_qk, "w_v": w_v, "w_out": w_out, "b_out": b_out}
        )
    return in_maps


def _run(inputs, trace=False):
    _ensure_paths()
    import os

    if trace:
        os.environ.pop("BASS_NEVER_TRACE", None)
    else:
        os.environ["BASS_NEVER_TRACE"] = "1"
    from concourse import bass_utils

    nc = _build()
    in_maps = _prep_inputs(inputs)
    res = bass_utils.run_bass_kernel_spmd(
        nc, in_maps, core_ids=list(range(N_CORES)), trace=trace
    )
    out = np.stack(
        [res.results[i]["out"].reshape(32, 32, DIM) for i in range(N_CORES)]
    ).astype(np.float32)
    return out, res


def kernel(**inputs):
    out, _ = _run(inputs, trace=False)
    return out
